# revision 3
# baseline (speedup 1.0000x reference)
"""AttnDecoderRNN fused single-step kernel for Trainium2 (Bass/Tile), 8-core
data parallel.

Problem (per reference):
  B=16384, H=512, S=6, O=1
  attn_in  = concat([input, hidden, cell])                    [B, 2H+O]
  attn_w   = softmax(attn_in @ W_attn.T + b_attn)             [B, S]
  attn_app = einsum('bs,bsh->bh', attn_w, encoder_outputs)    [B, H]
  x        = relu(concat([input, attn_app]) @ W_comb.T + b_comb)  [B, H]
  gates    = x @ W_ih.T + b_ih + hidden @ W_hh.T + b_hh       [B, 4H]
  i,f,g,o  = split(gates); c' = sig(f)*cell + sig(i)*tanh(g)
  h'       = sig(o)*tanh(c'); out = h' @ W_out.T + b_out
  returns (out, h', c', attn_w[:, None, :])

Sharding: batch split across 8 NeuronCores (2048 rows each); weights
replicated. Within a core: batch-major layout for elementwise/outputs; the
feature-contracted operands (hidden^T, cell^T, attn_app^T) are produced
on-chip with PE transposes; weights are pre-transposed on the host. Matmuls
run in float32r (full-rate fp32 PE mode, ~1e-4 rel err).
"""

import numpy as np

import concourse.bass as bass
import concourse.tile as tile
from concourse import bacc, mybir
from concourse import bass_utils
from concourse.masks import make_identity

F32 = mybir.dt.float32
F32R = mybir.dt.float32r
AF = mybir.ActivationFunctionType
ALU = mybir.AluOpType

B, H, S, O = 16384, 512, 6, 1
N_CORES = 8
BL = B // N_CORES          # 2048 rows per core
N_CHUNK = 4                # chunks per core
CB = BL // N_CHUNK         # 512 rows per chunk
N_BT = BL // 128           # 16 b-tiles per core
KO = H // 128              # 4 k-tiles over H


def build_kernel():
    nc = bacc.Bacc("TRN2", target_bir_lowering=False, debug=False,
                   num_devices=N_CORES)

    # --- DRAM I/O (per-core shapes; host pre-shards / pre-transposes) ---
    enc = nc.dram_tensor("enc", [BL, S, H], F32, kind="ExternalInput").ap()
    hid = nc.dram_tensor("hid", [BL, H], F32, kind="ExternalInput").ap()
    cel = nc.dram_tensor("cel", [BL, H], F32, kind="ExternalInput").ap()
    inp = nc.dram_tensor("inp", [N_CHUNK, CB], F32R, kind="ExternalInput").ap()
    wih = nc.dram_tensor("wih", [H, 4 * H], F32R, kind="ExternalInput").ap()   # W_ih.T
    whh = nc.dram_tensor("whh", [H, 4 * H], F32R, kind="ExternalInput").ap()   # W_hh.T
    wcm = nc.dram_tensor("wcm", [H, H], F32R, kind="ExternalInput").ap()       # W_comb[:,1:].T
    wc0 = nc.dram_tensor("wc0", [1, H], F32R, kind="ExternalInput").ap()       # W_comb[:,0]
    wah = nc.dram_tensor("wah", [H, S], F32R, kind="ExternalInput").ap()       # W_attn[:,1:513].T
    wac = nc.dram_tensor("wac", [H, S], F32R, kind="ExternalInput").ap()       # W_attn[:,513:].T
    wa0 = nc.dram_tensor("wa0", [1, S], F32R, kind="ExternalInput").ap()       # W_attn[:,0]
    ban = nc.dram_tensor("ban", [S, 1], F32, kind="ExternalInput").ap()        # b_attn
    bg = nc.dram_tensor("bg", [1, 4 * H], F32, kind="ExternalInput").ap()      # b_ih+b_hh
    bcm = nc.dram_tensor("bcm", [128, KO], F32, kind="ExternalInput").ap()     # b_comb tiled
    wo = nc.dram_tensor("wo", [1, H], F32, kind="ExternalInput").ap()          # W_out row
    one6 = nc.dram_tensor("one6", [S, 1], F32R, kind="ExternalInput").ap()     # ones

    hout = nc.dram_tensor("hout", [BL, H], F32, kind="ExternalOutput").ap()
    cout = nc.dram_tensor("cout", [BL, H], F32, kind="ExternalOutput").ap()
    awt = nc.dram_tensor("awt", [S, BL], F32, kind="ExternalOutput").ap()      # attn_w^T
    outv = nc.dram_tensor("outv", [128, N_BT], F32, kind="ExternalOutput").ap()

    wih3 = wih.rearrange("(ko p) j -> p ko j", p=128)
    whh3 = whh.rearrange("(ko p) j -> p ko j", p=128)
    wcm3 = wcm.rearrange("(ko p) h -> p ko h", p=128)
    wah3 = wah.rearrange("(ko p) s -> p ko s", p=128)
    wac3 = wac.rearrange("(ko p) s -> p ko s", p=128)

    with tile.TileContext(nc) as tc:
        with (
            tc.tile_pool(name="wpool", bufs=1) as wpool,
            tc.tile_pool(name="chunk", bufs=1) as chunk_pool,
            tc.tile_pool(name="io", bufs=2) as io,
            tc.tile_pool(name="io3", bufs=2) as io3,
            tc.tile_pool(name="celp", bufs=6) as celp,
            tc.tile_pool(name="ps_mm", bufs=1, space="PSUM") as ps_mm,
            tc.tile_pool(name="ps_tr", bufs=2, space="PSUM") as ps_tr,
            tc.tile_pool(name="ps_at", bufs=1, space="PSUM") as ps_at,
        ):
            # ---------------- one-time setup ----------------
            ident = wpool.tile([128, 128], F32)
            make_identity(nc, ident)
            ident6 = wpool.tile([S, S], F32)
            make_identity(nc, ident6)
            ones6 = wpool.tile([S, 1], F32R)
            nc.sync.dma_start(ones6[:], one6[:])

            wih_sb = wpool.tile([128, KO, 4 * H], F32R)
            nc.sync.dma_start(wih_sb[:], wih3[:])
            whh_sb = wpool.tile([128, KO, 4 * H], F32R)
            nc.sync.dma_start(whh_sb[:], whh3[:])
            wcm_sb = wpool.tile([128, KO, H], F32R)
            nc.sync.dma_start(wcm_sb[:], wcm3[:])
            wc0_sb = wpool.tile([1, H], F32R)
            nc.sync.dma_start(wc0_sb[:], wc0[:])
            wah_sb = wpool.tile([128, KO, S], F32R)
            nc.sync.dma_start(wah_sb[:], wah3[:])
            wac_sb = wpool.tile([128, KO, S], F32R)
            nc.sync.dma_start(wac_sb[:], wac3[:])
            wa0_sb = wpool.tile([1, S], F32R)
            nc.sync.dma_start(wa0_sb[:], wa0[:])
            ban_sb = wpool.tile([S, 1], F32)
            nc.sync.dma_start(ban_sb[:], ban[:])
            bcm_sb = wpool.tile([128, KO], F32)
            nc.sync.dma_start(bcm_sb[:], bcm[:])

            bg_row = io.tile([1, 4 * H], F32, tag="en", name="bg_row")
            nc.sync.dma_start(bg_row[:], bg[:])
            bg_bc = wpool.tile([128, 4 * H], F32)
            nc.gpsimd.partition_broadcast(bg_bc[:], bg_row[:])
            wo_row = io.tile([1, H], F32, tag="aa", name="wo_row")
            nc.sync.dma_start(wo_row[:], wo[:])
            wo_bc = wpool.tile([128, H], F32)
            nc.gpsimd.partition_broadcast(wo_bc[:], wo_row[:])

            outcols = wpool.tile([128, N_BT], F32)

            # ---------------- main loop over chunks ----------------
            for c in range(N_CHUNK):
                inp_row = chunk_pool.tile([1, CB], F32R, tag="inp_row")
                nc.sync.dma_start(inp_row[:], inp[c:c + 1, :])

                hT = chunk_pool.tile([128, KO, CB], F32R, tag="hT")
                cT = chunk_pool.tile([128, KO, CB], F32R, tag="cT")
                aT = chunk_pool.tile([128, KO, CB], F32R, tag="aT")
                xT = chunk_pool.tile([128, KO, CB], F32R, tag="xT")
                cns = []

                # --- transpose hidden & cell into feature-major ---
                for bt in range(N_CHUNK):
                    gbt = c * N_CHUNK + bt
                    hn = io.tile([128, H], F32, tag="hn", name="hn")
                    nc.sync.dma_start(hn[:], hid[gbt * 128:(gbt + 1) * 128, :])
                    cn = celp.tile([128, H], F32, tag="cn", name="cn")
                    nc.sync.dma_start(cn[:], cel[gbt * 128:(gbt + 1) * 128, :])
                    cns.append(cn)

                    tp1 = ps_tr.tile([128, 512], F32, tag="tps", name="tp1")
                    for ko in range(KO):
                        nc.tensor.transpose(
                            tp1[:, ko * 128:(ko + 1) * 128],
                            hn[:, ko * 128:(ko + 1) * 128], ident[:])
                    nc.vector.tensor_copy(
                        hT[:, :, bt * 128:(bt + 1) * 128],
                        tp1[:].rearrange("p (ko b) -> p ko b", ko=KO))

                    tp2 = ps_tr.tile([128, 512], F32, tag="tps", name="tp2")
                    for ko in range(KO):
                        nc.tensor.transpose(
                            tp2[:, ko * 128:(ko + 1) * 128],
                            cn[:, ko * 128:(ko + 1) * 128], ident[:])
                    nc.vector.tensor_copy(
                        cT[:, :, bt * 128:(bt + 1) * 128],
                        tp2[:].rearrange("p (ko b) -> p ko b", ko=KO))

                # --- attention logits^T [S, CB], softmax -> attn_w^T ---
                lg = ps_at.tile([S, CB], F32, tag="lgrs", name="lg")
                nc.tensor.matmul(lg[:], wa0_sb[:], inp_row[:],
                                 start=True, stop=False)
                for ko in range(KO):
                    nc.tensor.matmul(lg[:], wah_sb[:, ko, :], hT[:, ko, :],
                                     start=False, stop=False)
                for ko in range(KO):
                    nc.tensor.matmul(lg[:], wac_sb[:, ko, :], cT[:, ko, :],
                                     start=False, stop=(ko == KO - 1))
                expT = chunk_pool.tile([S, CB], F32R, tag="expT")
                nc.scalar.activation(expT[:], lg[:], AF.Exp, bias=ban_sb[:])

                rs = ps_at.tile([1, CB], F32, tag="lgrs", name="rs")
                nc.tensor.matmul(rs[:], ones6[:], expT[:], start=True, stop=True)
                rsum_i = chunk_pool.tile([1, CB], F32, tag="rsum_i")
                nc.vector.reciprocal(rsum_i[:], rs[:])
                rs6 = chunk_pool.tile([S, CB], F32, tag="rs6")
                nc.gpsimd.partition_broadcast(rs6[:], rsum_i[:])
                awT_sb = chunk_pool.tile([S, CB], F32, tag="awT_sb")
                nc.vector.tensor_mul(awT_sb[:], expT[:].bitcast(F32), rs6[:])
                nc.sync.dma_start(awt[:, c * CB:(c + 1) * CB], awT_sb[:])

                # --- attn_applied per b-tile (batch-major), then transpose ---
                for bt in range(N_CHUNK):
                    gbt = c * N_CHUNK + bt
                    aw_ps = ps_at.tile([128, S], F32, tag="aw", name="aw_ps")
                    nc.tensor.transpose(
                        aw_ps[:], awT_sb[:, bt * 128:(bt + 1) * 128], ident6[:])

                    en = io.tile([128, S, H], F32, tag="en", name="en")
                    nc.sync.dma_start(en[:], enc[gbt * 128:(gbt + 1) * 128, :, :])
                    aa = io.tile([128, H], F32, tag="aa", name="aa")
                    nc.vector.tensor_scalar_mul(aa[:], en[:, 0, :], aw_ps[:, 0:1])
                    for s in range(1, S):
                        nc.vector.scalar_tensor_tensor(
                            aa[:], en[:, s, :], aw_ps[:, s:s + 1], aa[:],
                            op0=ALU.mult, op1=ALU.add)

                    tp3 = ps_tr.tile([128, 512], F32, tag="tps", name="tp3")
                    for ko in range(KO):
                        nc.tensor.transpose(
                            tp3[:, ko * 128:(ko + 1) * 128],
                            aa[:, ko * 128:(ko + 1) * 128], ident[:])
                    nc.vector.tensor_copy(
                        aT[:, :, bt * 128:(bt + 1) * 128],
                        tp3[:].rearrange("p (ko b) -> p ko b", ko=KO))

                # --- comb: x^T = relu(W_comb @ cat^T + b_comb) ---
                for ht in range(KO):
                    cb_ps = ps_mm.tile([128, 512], F32, tag=f"psg{ht}",
                                       name="cb_ps")
                    nc.tensor.matmul(cb_ps[:],
                                     wc0_sb[0:1, ht * 128:(ht + 1) * 128],
                                     inp_row[:], start=True, stop=False)
                    for ko in range(KO):
                        nc.tensor.matmul(
                            cb_ps[:], wcm_sb[:, ko, ht * 128:(ht + 1) * 128],
                            aT[:, ko, :], start=False, stop=(ko == KO - 1))
                    nc.scalar.activation(xT[:, ht, :], cb_ps[:], AF.Relu,
                                         bias=bcm_sb[:, ht:ht + 1])

                # --- gates + LSTM elementwise per b-tile ---
                for bt in range(N_CHUNK):
                    gbt = c * N_CHUNK + bt
                    bsl = slice(bt * 128, (bt + 1) * 128)
                    psg = [ps_mm.tile([128, 512], F32, tag=f"psg{j}",
                                      name=f"psg{j}") for j in range(4)]
                    for ko in range(KO):
                        for j in range(4):
                            nc.tensor.matmul(
                                psg[j][:], xT[:, ko, bsl],
                                wih_sb[:, ko, j * 512:(j + 1) * 512],
                                start=(ko == 0), stop=False)
                    for ko in range(KO):
                        for j in range(4):
                            nc.tensor.matmul(
                                psg[j][:], hT[:, ko, bsl],
                                whh_sb[:, ko, j * 512:(j + 1) * 512],
                                start=False, stop=(ko == KO - 1))

                    gb = [io.tile([128, 512], F32, tag=f"gb{j}", name=f"gb{j}")
                          for j in range(4)]
                    for j in range(4):
                        nc.vector.tensor_add(gb[j][:], psg[j][:],
                                             bg_bc[:, j * 512:(j + 1) * 512])
                    # i, f, g, o activations (in place)
                    nc.scalar.activation(gb[0][:], gb[0][:], AF.Sigmoid)
                    nc.scalar.activation(gb[1][:], gb[1][:], AF.Sigmoid)
                    nc.scalar.activation(gb[2][:], gb[2][:], AF.Tanh)
                    nc.scalar.activation(gb[3][:], gb[3][:], AF.Sigmoid)

                    c_new = io3.tile([128, H], F32, tag="c_new", name="c_new")
                    # t1 = sig(i)*tanh(g) -> gb2 ; t2 = sig(f)*cell -> gb1
                    nc.vector.tensor_mul(gb[2][:], gb[0][:], gb[2][:])
                    nc.vector.tensor_mul(gb[1][:], gb[1][:], cns[bt][:])
                    nc.vector.tensor_add(c_new[:], gb[1][:], gb[2][:])
                    # tanh(c') -> gb0 ; h' = sig(o)*tanh(c')
                    nc.scalar.activation(gb[0][:], c_new[:], AF.Tanh)
                    h_new = io3.tile([128, H], F32, tag="h_new", name="h_new")
                    nc.vector.tensor_mul(h_new[:], gb[3][:], gb[0][:])

                    nc.sync.dma_start(cout[gbt * 128:(gbt + 1) * 128, :],
                                      c_new[:])
                    nc.sync.dma_start(hout[gbt * 128:(gbt + 1) * 128, :],
                                      h_new[:])

                    # out = h' . W_out (+ b_out on host)
                    ov = io.tile([128, H], F32, tag="aa", name="ov")
                    nc.vector.tensor_mul(ov[:], h_new[:], wo_bc[:])
                    nc.vector.reduce_sum(out=outcols[:, gbt:gbt + 1], in_=ov[:],
                                         axis=mybir.AxisListType.X)

            nc.sync.dma_start(outv[:], outcols[:])

    nc.compile()
    return nc


_NC_CACHE = None


def _get_nc():
    global _NC_CACHE
    if _NC_CACHE is None:
        _NC_CACHE = build_kernel()
    return _NC_CACHE


def make_in_maps(input, hidden, cell, encoder_outputs,
                 W_attn, b_attn, W_comb, b_comb,
                 W_ih, W_hh, b_ih, b_hh, W_out, b_out):
    f32 = np.float32
    input = np.asarray(input, f32)
    hidden = np.asarray(hidden, f32)
    cell = np.asarray(cell, f32)
    encoder_outputs = np.asarray(encoder_outputs, f32)
    W_attn = np.asarray(W_attn, f32)
    b_attn = np.asarray(b_attn, f32)
    W_comb = np.asarray(W_comb, f32)
    b_comb = np.asarray(b_comb, f32)
    W_ih = np.asarray(W_ih, f32)
    W_hh = np.asarray(W_hh, f32)
    W_out = np.asarray(W_out, f32)
    bg = (np.asarray(b_ih, f32) + np.asarray(b_hh, f32)).reshape(1, 4 * H)

    shared = {
        "wih": np.ascontiguousarray(W_ih.T),
        "whh": np.ascontiguousarray(W_hh.T),
        "wcm": np.ascontiguousarray(W_comb[:, 1:].T),
        "wc0": np.ascontiguousarray(W_comb[:, 0].reshape(1, H)),
        "wah": np.ascontiguousarray(W_attn[:, 1:H + 1].T),
        "wac": np.ascontiguousarray(W_attn[:, H + 1:].T),
        "wa0": np.ascontiguousarray(W_attn[:, 0].reshape(1, S)),
        "ban": np.ascontiguousarray(b_attn.reshape(S, 1)),
        "bg": bg,
        "bcm": np.ascontiguousarray(b_comb.reshape(KO, 128).T),
        "wo": np.ascontiguousarray(W_out.reshape(1, H)),
        "one6": np.ones((S, 1), f32),
    }
    in_maps = []
    for i in range(N_CORES):
        sl = slice(i * BL, (i + 1) * BL)
        m = dict(shared)
        m["enc"] = encoder_outputs[sl]
        m["hid"] = hidden[sl]
        m["cel"] = cell[sl]
        m["inp"] = np.ascontiguousarray(input[sl].reshape(N_CHUNK, CB))
        in_maps.append(m)
    return in_maps


def assemble_outputs(results, b_out):
    b_out = float(np.asarray(b_out).reshape(-1)[0])
    outs, hs, cs, aws = [], [], [], []
    for r in results:
        hs.append(r["hout"])
        cs.append(r["cout"])
        aws.append(np.ascontiguousarray(r["awt"].T))       # [BL, S]
        outs.append(r["outv"].T.reshape(BL, 1) + b_out)    # [BL, 1]
    output = np.concatenate(outs, 0).astype(np.float32)
    h_new = np.concatenate(hs, 0).astype(np.float32)
    c_new = np.concatenate(cs, 0).astype(np.float32)
    attn_w = np.concatenate(aws, 0).astype(np.float32).reshape(B, 1, S)
    return output, h_new, c_new, attn_w


def kernel(**inputs):
    nc = _get_nc()
    in_maps = make_in_maps(**inputs)
    res = bass_utils.run_bass_kernel_spmd(nc, in_maps,
                                          core_ids=list(range(N_CORES)))
    return assemble_outputs(res.results, inputs["b_out"])


if __name__ == "__main__":
    rng = np.random.default_rng(0)
    demo = {
        "input": rng.standard_normal((B, O)).astype(np.float32),
        "hidden": rng.standard_normal((B, H)).astype(np.float32),
        "cell": rng.standard_normal((B, H)).astype(np.float32),
        "encoder_outputs": rng.standard_normal((B, S, H)).astype(np.float32),
        "W_attn": (rng.standard_normal((S, 2 * H + O)) * 0.02).astype(np.float32),
        "b_attn": (rng.standard_normal(S) * 0.02).astype(np.float32),
        "W_comb": (rng.standard_normal((H, H + O)) * 0.02).astype(np.float32),
        "b_comb": (rng.standard_normal(H) * 0.02).astype(np.float32),
        "W_ih": (rng.standard_normal((4 * H, H)) * 0.02).astype(np.float32),
        "W_hh": (rng.standard_normal((4 * H, H)) * 0.02).astype(np.float32),
        "b_ih": (rng.standard_normal(4 * H) * 0.02).astype(np.float32),
        "b_hh": (rng.standard_normal(4 * H) * 0.02).astype(np.float32),
        "W_out": (rng.standard_normal((O, H)) * 0.02).astype(np.float32),
        "b_out": (rng.standard_normal(O) * 0.02).astype(np.float32),
    }
    outs = kernel(**demo)
    for o in outs:
        print(o.shape, o.dtype)
